# revision 4
# baseline (speedup 1.0000x reference)
"""Diagonally-masked multi-head self-attention on 8 TRN2 NeuronCores.

Sharding (hardcoded): core c -> batch c//4, head group c%4 (4 heads each).
Host sums the 4 partial output projections per batch (the "all-reduce").

Design (bf16 operands, fp32 PSUM accumulate; rel err 4.4e-3 vs 2e-2 gate):
  - bf16 inputs/weights/activations halve input DMA (10MB->5MB) and all
    DVE copy/mask costs vs the original fp32r kernel.
  - x streams L-chunk-major (32 pieces), so KT/QT/scores/exp for query
    chunk 0 start after ~1.3MB instead of 8MB.
  - steady-state rep pipeline: next rep's x is DMA-prefetched into a second
    xt tile set during this rep's second half; next rep's first KT/QT
    groups and V tiles run as fillers in this rep's last attention calls;
    the previous rep's chunk-3 norms+projections run as fillers under this
    rep's first exp stream ("carry").  ScalarE's exp stream (~134-147us/core,
    dtype-independent 1 elem/lane/cycle -- the irreducible load) is fed with
    minimal rep-boundary gaps.
  - scores transposed ST[k, q] in fp32 PSUM; softmax denominator from a
    ones column appended to V (row 64 of the PV accumulator); diagonal mask
    = multiply exp(scores) by (1-I) on the overlap block; 1/denominator
    broadcast across partitions with a K=1 PE matmul; output projection
    K=128 with odd heads DMA-shifted to partitions 64-127 (on the SP ring;
    NEVER the ACT HWDGE ring -- a dependent DMA there stalls every exp
    behind it in ACT's strict 8-deep FIFO, measured +100us).

Timing: axon relay walls are 50-100ms with +/-10ms multimodal jitter;
medians of blocking calls give garbage slopes (same bits measured
180/304/564us).  Use batched-async timing (dispatch ~25 calls, block once,
slope reps=6 vs 16): this kernel ~257us/rep vs ~279us/rep for the same
dtypes on the unrestructured schedule (paired diff -10us/rep +/-7);
cost-model marginal 196us/rep.  Original fp32r baseline: 289us (old method).
"""

import numpy as np

import concourse.bass as bass
import concourse.mybir as mybir
import concourse.tile as tile
from concourse import bacc
from concourse.bass_utils import run_bass_kernel_spmd

B, L, DIM = 2, 2048, 1024
H, D = 16, 64
NCORES = 8
HPC = 4  # heads per core
GCOLS = HPC * D  # 256 weight cols per core
KCH = DIM // 128  # 8 contraction chunks for the projections
QC = L // 512  # 4 query chunks
JT = L // 128  # 16 key tiles
SCALE = 1.0 / 8.0  # 1/sqrt(D)

F32 = mybir.dt.float32
F32R = mybir.dt.float32r
BF16 = mybir.dt.bfloat16
EXP = mybir.ActivationFunctionType.Exp

_NC_CACHE = {}


def _build_nc(reps=1):
    if reps in _NC_CACHE:
        return _NC_CACHE[reps]

    nc = bacc.Bacc("TRN2", target_bir_lowering=False, debug=False, num_devices=NCORES)

    xT_d = nc.dram_tensor("xT", [QC, DIM, 512], BF16, kind="ExternalInput")
    wq_d = nc.dram_tensor("wq", [DIM, GCOLS], BF16, kind="ExternalInput")
    wk_d = nc.dram_tensor("wk", [DIM, GCOLS], BF16, kind="ExternalInput")
    wv_d = nc.dram_tensor("wv", [DIM, GCOLS], BF16, kind="ExternalInput")
    wo_d = nc.dram_tensor("wo", [GCOLS, DIM], BF16, kind="ExternalInput")
    out_d = nc.dram_tensor("out", [L, DIM], F32, kind="ExternalOutput")
    diag_d = nc.inline_tensor(
        np.ascontiguousarray((1.0 - np.eye(128)).astype(mybir.dt.np(BF16))),
        name="diagmask",
    )

    with tile.TileContext(nc) as tc:
        with (
            tc.tile_pool(name="singles", bufs=1) as singles,
            tc.tile_pool(name="big", bufs=16) as big,
            tc.tile_pool(name="etp", bufs=6) as etp,
            tc.tile_pool(name="otn", bufs=6) as otnp,
            tc.tile_pool(name="tmpp", bufs=2) as tmpp,
            tc.tile_pool(name="osb", bufs=3) as outp,
            tc.tile_pool(name="rd", bufs=6) as rdp,
            tc.tile_pool(name="bp", bufs=2, space="PSUM") as bp,
            tc.tile_pool(name="otps", bufs=2, space="PSUM") as otp,
            tc.tile_pool(name="smp", bufs=2, space="PSUM") as smp,
        ):
            # ---- static loads -------------------------------------------
            wq_t = singles.tile([128, KCH, GCOLS], BF16, tag="wq")
            wk_t = singles.tile([128, KCH, GCOLS], BF16, tag="wk")
            wv_t = singles.tile([128, KCH, GCOLS], BF16, tag="wv")
            wo_t = singles.tile([128, 2, DIM], BF16, tag="wo")
            diag_t = singles.tile([128, 128], BF16, tag="diag")
            ones_t = singles.tile([128, 64], F32R, tag="ones")
            vaug = singles.tile([128, JT, HPC, D + 1], BF16, tag="vaug")
            qt = [singles.tile([128, L], BF16, tag=f"qt{p}", name=f"qt{p}") for p in range(2)]
            kt = [singles.tile([128, L], BF16, tag=f"kt{p}", name=f"kt{p}") for p in range(2)]

            # pair-0 qk weights first (gate the whole pipeline), then the
            # rest ships between the early x chunks.
            nc.sync.dma_start(
                out=wq_t[:, :, 0:128],
                in_=wq_d[:, 0:128].rearrange("(c p) n -> p c n", p=128),
            )
            nc.sync.dma_start(
                out=wk_t[:, :, 0:128],
                in_=wk_d[:, 0:128].rearrange("(c p) n -> p c n", p=128),
            )
            nc.sync.dma_start(out=diag_t, in_=diag_d[:])
            nc.vector.memset(ones_t[:].bitcast(F32), 1.0)
            nc.vector.memset(vaug[:, :, :, D], 1.0)

            def xdma(xt, c4, ks=range(KCH)):
                for k in ks:
                    nc.sync.dma_start(
                        out=xt[k][:, 512 * c4 : 512 * (c4 + 1)],
                        in_=xT_d[c4, 128 * k : 128 * (k + 1), :],
                    )

            def new_xt(r):
                return [
                    big.tile([128, L], BF16, tag="big", name=f"xt{r}_{k}")
                    for k in range(KCH)
                ]

            # rep 0 head: x chunk 0 -> wv -> chunk 1 -> pair-1 qk weights ->
            # chunk 2 -> wo -> chunk 3
            xt = new_xt(0)
            xdma(xt, 0)
            nc.sync.dma_start(out=wv_t, in_=wv_d[:].rearrange("(c p) n -> p c n", p=128))
            xdma(xt, 1)
            nc.sync.dma_start(
                out=wq_t[:, :, 128:256],
                in_=wq_d[:, 128:256].rearrange("(c p) n -> p c n", p=128),
            )
            nc.sync.dma_start(
                out=wk_t[:, :, 128:256],
                in_=wk_d[:, 128:256].rearrange("(c p) n -> p c n", p=128),
            )
            xdma(xt, 2)
            nc.sync.dma_start(out=wo_t, in_=wo_d[:].rearrange("(g p) n -> p g n", p=128))
            xdma(xt, 3)

            otn = {}
            carry = []  # prev rep's tail work, placed in this rep's first call
            for rep in range(reps):
                xt_next = None

                def qk_group(pair, qk, c4, src=None, pool=None, tag=None):
                    """One [128, 512] accumulation group of QT or KT."""
                    wt, dst = ((wq_t, qt[pair]), (wk_t, kt[pair]))[qk]
                    x_src = src if src is not None else xt
                    p, tg = (pool or smp), (tag or "sm")
                    ps = p.tile([128, 512], F32, tag=tg, name=f"ps{'qk'[qk]}{pair}_{c4}")
                    for k in range(KCH):
                        nc.tensor.matmul(
                            out=ps,
                            lhsT=wt[:, k, 128 * pair : 128 * (pair + 1)],
                            rhs=x_src[k][:, 512 * c4 : 512 * (c4 + 1)],
                            start=(k == 0),
                            stop=(k == KCH - 1),
                        )
                    nc.vector.tensor_copy(out=dst[:, 512 * c4 : 512 * (c4 + 1)], in_=ps)

                def v_group(t, pool=None, tag=None, src=None):
                    p, tg = (pool or smp), (tag or "sm")
                    x_src = src if src is not None else xt
                    ps = p.tile([128, GCOLS], F32, tag=tg, name=f"psv{t}")
                    for k in range(KCH):
                        nc.tensor.matmul(
                            out=ps,
                            lhsT=x_src[k][:, 128 * t : 128 * (t + 1)],
                            rhs=wv_t[:, k, :],
                            start=(k == 0),
                            stop=(k == KCH - 1),
                        )
                    nc.vector.tensor_copy(
                        out=vaug[:, t, :, 0:D],
                        in_=ps[:].rearrange("p (h d) -> p h d", h=HPC),
                    )

                def attn(c, pair, pre_av=None, extras=None):
                    # extras: list of (j_slot, thunk) placed inside the j loop
                    positions = {}
                    for js, th in extras or []:
                        positions.setdefault(js, []).append(th)
                    ha, hb = 2 * pair, 2 * pair + 1
                    ot_a = otp.tile([D + 1, 512], F32, tag="ot", name=f"ota{c}_{pair}")
                    ot_b = otp.tile([D + 1, 512], F32, tag="ot", name=f"otb{c}_{pair}")
                    for j in range(JT):
                        st = bp.tile([128, 1024], F32, tag="bp", name=f"st{c}_{pair}_{j}")
                        # scores (transposed): ST[k-tile, q-chunk]; the two heads
                        # of the pair run concurrently via row tiling.
                        nc.tensor.matmul(
                            out=st[:, 0:512],
                            lhsT=kt[pair][0:64, 128 * j : 128 * (j + 1)],
                            rhs=qt[pair][0:64, 512 * c : 512 * (c + 1)],
                            start=True,
                            stop=True,
                        )
                        nc.tensor.matmul(
                            out=st[:, 512:1024],
                            lhsT=kt[pair][64:128, 128 * j : 128 * (j + 1)],
                            rhs=qt[pair][64:128, 512 * c : 512 * (c + 1)],
                            start=True,
                            stop=True,
                        )
                        et = etp.tile([128, 1024], BF16, tag="et", name=f"et{c}_{pair}_{j}")
                        nc.scalar.activation(out=et, in_=st, func=EXP, scale=SCALE)
                        if 4 * c <= j < 4 * (c + 1):
                            off = 128 * (j - 4 * c)
                            nc.vector.tensor_mul(
                                out=et[:, off : off + 128],
                                in0=et[:, off : off + 128],
                                in1=diag_t,
                            )
                            nc.vector.tensor_mul(
                                out=et[:, 512 + off : 512 + off + 128],
                                in0=et[:, 512 + off : 512 + off + 128],
                                in1=diag_t,
                            )
                        if pre_av is not None:
                            pre_av(j)
                        for th in positions.get(j, []):
                            th()
                        # PV (+ denominator in row 64 via the ones column)
                        nc.tensor.matmul(
                            out=ot_a,
                            lhsT=vaug[:, j, ha, :],
                            rhs=et[:, 0:512],
                            start=(j == 0),
                            stop=(j == JT - 1),
                        )
                        nc.tensor.matmul(
                            out=ot_b,
                            lhsT=vaug[:, j, hb, :],
                            rhs=et[:, 512:1024],
                            start=(j == 0),
                            stop=(j == JT - 1),
                        )

                    def norm_half(h, ot, top):
                        def run():
                            # one copy frees the PSUM accumulator bank fast, so
                            # the next call's first PV matmul isn't queued
                            # behind the whole normalize chain
                            otf = rdp.tile([D + 1, 512], F32R, tag="rd", name=f"otf{c}_{h}")
                            nc.vector.tensor_copy(out=otf, in_=ot)
                            rd = otf  # reciprocal written back into row 64
                            with nc.allow_low_precision(reason="1/D rounded to fp32r"):
                                nc.vector.reciprocal(
                                    out=rd[D : D + 1, :], in_=otf[D : D + 1, :]
                                )
                            # broadcast 1/D (partition 64) to 64 partitions via PE
                            rdb_ps = smp.tile([D, 512], F32, tag="sm", name=f"rdps{c}_{h}")
                            nc.tensor.matmul(
                                out=rdb_ps,
                                lhsT=ones_t[D : D + 1, :],
                                rhs=rd[D : D + 1, :],
                                start=True,
                                stop=True,
                            )
                            rdb = rdp.tile([D, 512], F32, tag="rd", name=f"rdb{c}_{h}")
                            nc.vector.tensor_copy(out=rdb, in_=rdb_ps)
                            if top:
                                # heads 0/2 land on partitions 0-63 of the paired tile
                                otn2 = otnp.tile(
                                    [128, 512], BF16, tag="otn", name=f"otn{c}_{pair}"
                                )
                                otn[(pair, c)] = otn2
                                nc.vector.tensor_mul(
                                    out=otn2[0:D, :], in0=otf[0:D, :], in1=rdb[:]
                                )
                            else:
                                # heads 1/3: normalize then DMA-shift to partitions 64-127
                                tmp = tmpp.tile([D, 512], BF16, tag="tmp", name=f"otmp{c}_{pair}")
                                nc.vector.tensor_mul(out=tmp, in0=otf[0:D, :], in1=rdb[:])
                                nc.sync.dma_start(out=otn[(pair, c)][D : 2 * D, :], in_=tmp)

                        return run

                    return [norm_half(ha, ot_a, True), norm_half(hb, ot_b, False)]

                def proj_group(c, tt, half):
                    t = 4 * c + tt
                    onp = smp.tile([128, 512], F32, tag="sm", name=f"onp{t}_{half}")
                    for g in range(2):
                        nc.tensor.matmul(
                            out=onp,
                            lhsT=otn[(g, c)][:, 128 * tt : 128 * (tt + 1)],
                            rhs=wo_t[:, g, 512 * half : 512 * (half + 1)],
                            start=(g == 0),
                            stop=(g == 1),
                        )
                    osb = outp.tile([128, 512], F32, tag="osb", name=f"osb{t}_{half}")
                    nc.vector.tensor_copy(out=osb, in_=onp)
                    nc.sync.dma_start(
                        out=out_d[128 * t : 128 * (t + 1), 512 * half : 512 * (half + 1)],
                        in_=osb,
                    )

                def proj_thunks(c):
                    return [
                        (lambda tt=tt, half=half: proj_group(c, tt, half))
                        for tt in range(4)
                        for half in range(2)
                    ]

                def qk_thunk(pair, qk, c4, **kw):
                    return lambda: qk_group(pair, qk, c4, **kw)

                def spread(*groups):
                    """Interleave thunk groups into j-slots 2..15."""
                    ex = []
                    slots = [3, 5, 7, 9, 11, 13, 15, 2, 4, 6, 8, 10, 12, 14]
                    i = 0
                    for g in groups:
                        for th in g:
                            ex.append((slots[i % len(slots)], th))
                            i += 1
                    return ex

                def norms_at(norms):
                    return [(0, norms[0]), (1, norms[1])] if norms else []

                last = rep == reps - 1

                # prefetch emitters for the next rep's x (c4-major pieces)
                def prefetch(c4):
                    def run():
                        xdma(xt_next, c4)

                    return run

                # ---- emission ------------------------------------------------
                # head (rep 0 only; for rep>0 these ran as fillers in the
                # previous rep's tail calls)
                if rep == 0:
                    qk_group(0, 1, 0)  # KT pair 0, key tiles 0-3
                    qk_group(0, 0, 0)  # QT pair 0, chunk 0
                    v_group(0, pool=otp, tag="ot")
                    first_av = lambda j: v_group(j + 1) if j < JT - 1 else None
                    first_extras = [(2, qk_thunk(0, 1, 1)),
                                    (6, qk_thunk(0, 1, 2)),
                                    (10, qk_thunk(0, 1, 3)),
                                    (13, qk_thunk(0, 0, 1))]
                else:
                    # v(1..5), kt0(c1..3) ran in the previous rep's tail calls;
                    # the previous rep's chunk-3 norms+projections run here as
                    # fillers under this rep's first exp stream
                    first_av = lambda j: v_group(j + 1) if 5 <= j < JT - 1 else None
                    first_extras = carry + [(13, qk_thunk(0, 0, 1))]

                n00 = attn(0, 0, pre_av=first_av, extras=first_extras)
                n10 = attn(1, 0, extras=norms_at(n00) + spread(
                    [qk_thunk(1, 1, 0), qk_thunk(1, 1, 1), qk_thunk(0, 0, 2)]))
                n20 = attn(2, 0, extras=norms_at(n10) + spread(
                    [qk_thunk(1, 1, 2), qk_thunk(1, 1, 3), qk_thunk(0, 0, 3)]))
                if not last:
                    xt_next = new_xt(rep + 1)
                n30 = attn(3, 0, extras=norms_at(n20) + spread(
                    [qk_thunk(1, 0, 0)] + ([prefetch(0)] if not last else [])))
                n01 = attn(0, 1, extras=norms_at(n30) + spread(
                    [qk_thunk(1, 0, 1)]
                    + ([qk_thunk(0, 1, 0, src=xt_next), prefetch(1)] if not last else [])))
                n11 = attn(1, 1, extras=norms_at(n01) + spread(
                    [qk_thunk(1, 0, 2)],
                    proj_thunks(0),
                    [qk_thunk(0, 1, 1, src=xt_next), prefetch(2)] if not last else []))
                n21 = attn(2, 1, extras=norms_at(n11) + spread(
                    [qk_thunk(1, 0, 3)],
                    proj_thunks(1),
                    [qk_thunk(0, 1, 2, src=xt_next), prefetch(3)] if not last else []))
                # next rep's remaining KT/QT groups + first V tiles run here so
                # its exp stream starts right after this rep's last PV
                if not last:
                    nxt = [(3, qk_thunk(0, 1, 3, src=xt_next)),
                           (5, qk_thunk(0, 0, 0, src=xt_next))] + [
                        (t + 2, (lambda t=t: v_group(t, src=xt_next)))
                        for t in range(6)
                    ]
                else:
                    nxt = []
                n31 = attn(3, 1, extras=norms_at(n21) + nxt + spread(
                    proj_thunks(2)))
                xt = xt_next
                if last:
                    for th in n31:
                        th()
                    for th in proj_thunks(3):
                        th()
                else:
                    carry = norms_at(n31) + spread(proj_thunks(3))

    nc.compile()
    _NC_CACHE[reps] = nc
    return nc


def make_in_maps(x, Wq, Wk, Wv, Wo):
    np_bf16 = mybir.dt.np(BF16)
    x = np.asarray(x, dtype=np.float32)
    Wq = np.asarray(Wq, dtype=np_bf16)
    Wk = np.asarray(Wk, dtype=np_bf16)
    Wv = np.asarray(Wv, dtype=np_bf16)
    Wo = np.asarray(Wo, dtype=np_bf16)
    in_maps = []
    for core in range(NCORES):
        b, g = core // HPC, core % HPC
        cs = slice(GCOLS * g, GCOLS * (g + 1))
        xt = x[b].T.astype(np_bf16)  # [DIM, L]
        xlm = np.ascontiguousarray(
            xt.reshape(DIM, QC, 512).transpose(1, 0, 2)
        )  # [QC, DIM, 512]
        in_maps.append(
            {
                "xT": xlm,
                "wq": np.ascontiguousarray(Wq[:, cs]),
                "wk": np.ascontiguousarray(Wk[:, cs]),
                "wv": np.ascontiguousarray(Wv[:, cs]),
                "wo": np.ascontiguousarray(Wo[cs, :]),
            }
        )
    return in_maps


def combine_outputs(results):
    out = np.zeros((B, L, DIM), dtype=np.float32)
    for core in range(NCORES):
        out[core // HPC] += results[core]["out"]
    return out


def kernel(x, Wq, Wk, Wv, Wo):
    nc = _build_nc()
    in_maps = make_in_maps(x, Wq, Wk, Wv, Wo)
    last_err = None
    for _ in range(3):
        try:
            res = run_bass_kernel_spmd(nc, in_maps, core_ids=list(range(NCORES)))
            return combine_outputs(res.results)
        except Exception as e:  # transient NRT/device-unrecoverable states
            last_err = e
    raise last_err
